# revision 1
# baseline (speedup 1.0000x reference)
"""Trainium2 Bass kernel for nn_BasicBlock (FBS-masked ternary conv + BN + LeakyReLU).

Sharding: data-parallel over batch. B=32 -> 4 samples per core on 8 cores.
Weight/saliency prep is replicated per-core; BN batch stats are AllReduced.

Math notes:
  - tw = pos*[W>t] + neg*[W<-t], t = 0.05*max|W|.  We compute w' = tw/pos =
    [W>t] + r*[W<-t] (r = neg/pos, host immediate) so the conv output is
    y' = y/pos.  BN normalization is invariant to that scale except through
    eps: we use eps' = eps/pos^2 (host immediate).  pos > 0 always.
  - Conv (stride 2, pad 1, K=4) = 32 accumulating f32r matmuls per output
    tile: 2 ci-tiles x 16 kernel offsets, x stored as 4 parity quadrants
    Qd[ph][pw][ci, j, i] = x[ci, 2j+ph, 2i+pw].  Padding is handled by
    restricting the (oh, ow) range of boundary offsets; the full-coverage
    (kh=1, kw=1) matmul goes first with start=True.
  - Top-k threshold (k=410 of 512) is exact: count matrix via compare +
    ones-matmul, thr = min{s_j : #{s_i > s_j} <= 409}.
"""

import numpy as np

import concourse.bass as bass
import concourse.mybir as mybir
import concourse.tile as tile
from concourse.bass_utils import run_bass_kernel_spmd
from concourse.masks import make_identity

F32 = mybir.dt.float32
F32R = mybir.dt.float32r
AF = mybir.ActivationFunctionType
ALU = mybir.AluOpType
AX = mybir.AxisListType

N_CORES = 8
B, CIN, H, W = 32, 256, 64, 64
COUT, KK = 512, 4
OH, OW = 32, 32
NB = B // N_CORES          # samples per core = 4
NT = CIN // 128            # ci tiles = 2
NCOT = COUT // 128         # co tiles = 4
CR_KEEP = 409.5            # count <= 409  <->  count < 409.5
BN_EPS = 1e-5
NEG_SLOPE = 0.2
THRESH_FACTOR = 0.05
NSP = OH * OW              # 1024 spatial positions per sample

MAX_WAITS = 1              # this walrus build allows 1 sync wait per instruction

# kh -> (row parity ph, row shift dj): x row 2*oh + kh - 1 = 2*(oh+dj) + ph
PAR = {0: (1, -1), 1: (0, 0), 2: (1, 0), 3: (0, 1)}
KHW_ORDER = [(1, 1)] + [(kh, kw) for kh in range(KK) for kw in range(KK)
                        if (kh, kw) != (1, 1)]


def _split_waits(nc, max_waits=MAX_WAITS):
    """Split per-instruction sem waits exceeding max_waits into preceding
    same-engine InstNoOp carriers (engines execute their queue in order)."""
    for f in nc.m.functions:
        for bb in f.blocks:
            new_list = []
            changed = False
            for ins in bb.instructions:
                si = ins.sync_info
                if si is not None and si.on_wait and len(si.on_wait) > max_waits:
                    waits = list(si.on_wait)
                    carry = waits[: len(waits) - max_waits]
                    keep = waits[len(waits) - max_waits:]
                    k = 0
                    while carry:
                        chunk, carry = carry[:max_waits], carry[max_waits:]
                        new_list.append(
                            mybir.InstNoOp(
                                name=f"{ins.name}_ws{k}",
                                engine=ins.engine,
                                bass_nofuse=True,
                                sync_info=mybir.SyncInfo(on_wait=chunk, on_update=[]),
                            )
                        )
                        k += 1
                    ins.sync_info = mybir.SyncInfo(
                        on_wait=keep, on_update=list(si.on_update)
                    )
                    changed = True
                new_list.append(ins)
            if changed:
                bb.instructions = new_list


def build_kernel(r_imm: float, eps_imm: float, debug: bool = False):
    """Build the per-core Bass module. r_imm = neg/pos, eps_imm = eps/pos^2."""
    nc = bass.Bass()

    xs = nc.dram_tensor("xs", [NB, CIN, H, W], F32, kind="ExternalInput")
    wt = nc.dram_tensor("wt", [COUT, CIN, KK, KK], F32, kind="ExternalInput")
    salw = nc.dram_tensor("salw", [COUT, CIN], F32, kind="ExternalInput")
    salb = nc.dram_tensor("salb", [COUT], F32, kind="ExternalInput")
    gam = nc.dram_tensor("gam", [COUT], F32, kind="ExternalInput")
    bet = nc.dram_tensor("bet", [COUT], F32, kind="ExternalInput")
    out = nc.dram_tensor("out", [NB, COUT, OH, OW], F32, kind="ExternalOutput")

    cc_out = nc.dram_tensor("cc_out", [2, NCOT, 128], F32, addr_space="Shared")

    if debug:
        dbg_mask = nc.dram_tensor("dbg_mask", [NB, COUT], F32, kind="ExternalOutput")
        dbg_sal = nc.dram_tensor("dbg_sal", [NB, COUT], F32, kind="ExternalOutput")
        dbg_sub = nc.dram_tensor("dbg_sub", [128, NT * NB], F32, kind="ExternalOutput")
        dbg_wq = nc.dram_tensor("dbg_wq", [NT, 128, 16 * COUT], F32, kind="ExternalOutput")
        dbg_y = nc.dram_tensor("dbg_y", [NB, NCOT, 128, NSP], F32, kind="ExternalOutput")
        dbg_stats = nc.dram_tensor("dbg_stats", [2, NCOT, 128], F32, kind="ExternalOutput")
        dbg_cnt = nc.dram_tensor("dbg_cnt", [NB, COUT], F32, kind="ExternalOutput")
        dbg_bc = nc.dram_tensor("dbg_bc", [NB, COUT], F32, kind="ExternalOutput")
        dbg_thr = nc.dram_tensor("dbg_thr", [NB, 1], F32, kind="ExternalOutput")

    with tile.TileContext(nc) as tc:
        with (
            tc.tile_pool(name="persist", bufs=1) as pp,
            tc.tile_pool(name="stage", bufs=2) as stp,
            tc.tile_pool(name="wtmp", bufs=3) as wtp,
            tc.tile_pool(name="evict", bufs=3) as evp,
            tc.tile_pool(name="topk", bufs=4) as tkp,
            tc.tile_pool(name="qpool", bufs=1) as qp,
            tc.tile_pool(name="psA", bufs=6, space="PSUM") as psA,
            tc.tile_pool(name="psS", bufs=2, space="PSUM") as psS,
            tc.tile_pool(name="dram", bufs=1, space="DRAM") as dp,
        ):
            # ---------- constants / small loads ----------
            ident = pp.tile([128, 128], F32, name="ident")
            make_identity(nc, ident)
            onesk = pp.tile([128, NB], F32, name="onesk")
            nc.vector.memset(onesk, 1.0)

            def col128(dram_vec, nm):  # [512] dram -> [128,4] sbuf (p=ch%128, c=ch//128)
                t_ = pp.tile([128, NCOT], F32, name=nm)
                ap = bass.AP(tensor=dram_vec, offset=0, ap=[[1, 128], [128, NCOT]])
                nc.sync.dma_start(out=t_, in_=ap)
                return t_

            salb_t = col128(salb, "salb_t")
            gam_t = col128(gam, "gam_t")
            bet_t = col128(bet, "bet_t")

            # ---------- weight prep: ternarize to w' = [W>t] + r*[W<-t] ----------
            # wq[t][ci, khw*512 + co] = w'[co, t*128+ci, kh, kw], khw = kh*4+kw
            wq = [pp.tile([128, 16 * COUT], F32R, name=f"wq{t}") for t in range(NT)]
            NCH = 4  # co chunks of 128 per ci tile
            mx = pp.tile([128, NT * NCH], F32, name="mx")

            def w_chunk_ap(t, c):
                return wt[c * 128:(c + 1) * 128,
                          t * 128:(t + 1) * 128, :, :].rearrange(
                              "co ci kh kw -> ci co (kh kw)")

            for t in range(NT):
                for c in range(NCH):
                    ch = stp.tile([128, 128, 16], F32, name=f"wch{t}{c}", tag="stage")
                    nc.sync.dma_start(out=ch, in_=w_chunk_ap(t, c))
                    nc.vector.tensor_reduce(
                        mx[:, t * NCH + c: t * NCH + c + 1], ch, axis=AX.XY,
                        op=ALU.max, apply_absolute_value=True)
            mxr = pp.tile([128, 1], F32, name="mxr")
            nc.vector.tensor_reduce(mxr, mx, axis=AX.X, op=ALU.max)
            ptr0 = psS.tile([128, 512], F32, name="ptr0", tag="small")
            nc.tensor.transpose(ptr0[0:1, 0:128], mxr, ident)
            gmaxrow = pp.tile([1, 128], F32, name="gmaxrow")
            nc.scalar.copy(gmaxrow, ptr0[0:1, 0:128])
            gmax = pp.tile([1, 1], F32, name="gmax")
            nc.vector.tensor_reduce(gmax, gmaxrow, axis=AX.X, op=ALU.max)
            tval = pp.tile([1, 1], F32, name="tval")
            nc.vector.tensor_scalar(tval, gmax, float(THRESH_FACTOR), None,
                                    op0=ALU.mult)
            t_d = dp.tile([1, 1], F32, name="t_d")
            nc.sync.dma_start(out=t_d, in_=tval)
            tcol = pp.tile([128, 1], F32, name="tcol")
            nc.sync.dma_start(
                out=tcol, in_=bass.AP(tensor=t_d.tensor, offset=t_d.offset,
                                      ap=[[0, 128], [1, 1]]))
            ntcol = pp.tile([128, 1], F32, name="ntcol")
            nc.vector.tensor_scalar(ntcol, tcol, -1.0, None, op0=ALU.mult)

            for t in range(NT):
                for c in range(NCH):
                    ch = stp.tile([128, 128, 16], F32, name=f"wcb{t}{c}", tag="stage")
                    nc.sync.dma_start(out=ch, in_=w_chunk_ap(t, c))
                    g1 = wtp.tile([128, 128, 16], F32, name=f"g1_{t}{c}", tag="wg")
                    nc.vector.tensor_scalar(g1, ch, tcol[:, :], None, op0=ALU.is_gt)
                    g2 = wtp.tile([128, 128, 16], F32, name=f"g2_{t}{c}", tag="wg")
                    nc.vector.tensor_scalar(g2, ch, ntcol[:, :], float(r_imm),
                                            op0=ALU.is_lt, op1=ALU.mult)
                    dst = bass.AP(tensor=wq[t].tensor, offset=wq[t].offset + c * 128,
                                  ap=[wq[t].ap[0], [1, 128], [COUT, 16]])
                    nc.vector.tensor_tensor(out=dst, in0=g1, in1=g2, op=ALU.add)

            # ---------- x load, quadrant interleave, |x| row sums ----------
            # Column dim padded to 34 (zero cols at i=-1 and i=32) so every
            # f32r matmul has an even 32-wide inner dim; row boundaries are
            # handled with sub-range matmuls (odd row counts are legal).
            quads = [[[[qp.tile([128, OH, OW + 2], F32R, name=f"q{b01}{t}{ph}{pw}")
                        for pw in range(2)] for ph in range(2)]
                      for t in range(NT)] for b01 in range(2)]
            zc = pp.tile([128, OH], F32, name="zc")
            nc.vector.memset(zc, 0.0)
            for b01 in range(2):
                for t in range(NT):
                    for ph in range(2):
                        for pw in range(2):
                            qt = quads[b01][t][ph][pw]
                            nc.vector.tensor_copy(
                                qt[:, :, 0:1].rearrange("p a b -> p (a b)"), zc)
                            nc.vector.tensor_copy(
                                qt[:, :, OW + 1:OW + 2].rearrange("p a b -> p (a b)"),
                                zc)
            subT = [pp.tile([128, NB], F32, name=f"subT{t}") for t in range(NT)]

            def load_sample(b, do_stats, do_quads):
                b01 = b % 2
                for t in range(NT):
                    half_sums = []
                    for hh in range(2):
                        stg = stp.tile([128, 32, W], F32,
                                       name=f"x{b}{t}{hh}{int(do_quads)}",
                                       tag="stage")
                        nc.sync.dma_start(
                            out=stg,
                            in_=xs[b, t * 128:(t + 1) * 128,
                                   hh * 32:(hh + 1) * 32, :])
                        if do_stats:
                            sh = evp.tile([128, 1], F32, name=f"sh{b}{t}{hh}",
                                          tag="subh", bufs=4)
                            nc.vector.tensor_reduce(sh, stg, axis=AX.XY, op=ALU.add,
                                                    apply_absolute_value=True)
                            half_sums.append(sh)
                        if do_quads:
                            for ph in range(2):
                                for pw in range(2):
                                    nc.vector.tensor_copy(
                                        quads[b01][t][ph][pw][
                                            :, hh * 16:(hh + 1) * 16, 1:OW + 1],
                                        stg[:, ph::2, pw::2])
                    if do_stats:
                        nc.vector.tensor_tensor(out=subT[t][:, b:b + 1],
                                                in0=half_sums[0], in1=half_sums[1],
                                                op=ALU.add)

            # samples 0,1: stats + quads from one load; samples 2,3: stats only
            # (their quads are re-loaded + interleaved during the pair-0 conv so
            # the saliency/top-k for ALL samples is available before eviction).
            load_sample(0, True, True)
            load_sample(1, True, True)
            load_sample(2, True, False)
            load_sample(3, True, False)

            # ---------- saliency + exact top-k threshold + mask ----------
            subm = [pp.tile([128, NB], F32, name=f"subm{t}") for t in range(NT)]
            for t in range(NT):
                nc.vector.tensor_scalar(subm[t], subT[t], 1.0 / (H * W), None,
                                        op0=ALU.mult)
            salwT = [pp.tile([128, COUT], F32, name=f"swT{t}") for t in range(NT)]
            for cot in range(NCOT):
                swn = stp.tile([128, CIN], F32, name=f"swn{cot}", tag="stage")
                nc.sync.dma_start(out=swn, in_=salw[cot * 128:(cot + 1) * 128, :])
                for t in range(NT):
                    ptr1 = psS.tile([128, 512], F32, name=f"ptw{cot}{t}", tag="small")
                    nc.tensor.transpose(ptr1[:, 0:128],
                                        swn[:, t * 128:(t + 1) * 128], ident)
                    nc.scalar.copy(salwT[t][:, cot * 128:(cot + 1) * 128],
                                   ptr1[:, 0:128])

            sal_cb = []
            for cot in range(NCOT):
                psal = psS.tile([128, 512], F32, name=f"psal{cot}", tag="small")
                for t in range(NT):
                    nc.tensor.matmul(psal[:, 0:NB],
                                     salwT[t][:, cot * 128:(cot + 1) * 128],
                                     subm[t], start=(t == 0), stop=(t == NT - 1))
                sc = pp.tile([128, NB], F32, name=f"salcb{cot}")
                nc.scalar.activation(sc, psal[:, 0:NB], AF.Abs,
                                     bias=salb_t[:, cot:cot + 1], scale=1.0)
                sal_cb.append(sc)

            salT = pp.tile([NB, COUT], F32, name="salT")
            for cot in range(NCOT):
                ptr2 = psS.tile([128, 512], F32, name=f"pts{cot}", tag="small")
                nc.tensor.transpose(ptr2[0:NB, 0:128], sal_cb[cot], ident)
                nc.scalar.copy(salT[:, cot * 128:(cot + 1) * 128],
                               ptr2[0:NB, 0:128])
            salT_d = dp.tile([NB, COUT], F32, name="salT_d")
            nc.sync.dma_start(out=salT_d, in_=salT)
            thr_d = dp.tile([NB, 1], F32, name="thr_d")

            BIG = 1.0e30
            for b in range(NB):
                bc = stp.tile([128, COUT], F32, name=f"bc{b}", tag="bc", bufs=1)
                nc.sync.dma_start(
                    out=bc, in_=bass.AP(tensor=salT_d.tensor,
                                        offset=salT_d.offset + b * COUT,
                                        ap=[[0, 128], [1, COUT]]))
                pC = psS.tile([128, 512], F32, name=f"pC{b}", tag="small")
                for cot in range(NCOT):
                    cmp = wtp.tile([128, COUT], F32, name=f"cmp{b}{cot}", tag="wg")
                    nc.vector.tensor_scalar(cmp, bc, sal_cb[cot][:, b:b + 1],
                                            None, op0=ALU.is_lt)
                    nc.tensor.matmul(pC[0:NB, :], onesk, cmp,
                                     start=(cot == 0), stop=(cot == NCOT - 1))
                Ct = tkp.tile([1, COUT], F32, name=f"Ct{b}", tag="tk")
                nc.scalar.copy(Ct, pC[0:1, :])
                m01 = tkp.tile([1, COUT], F32, name=f"m01{b}", tag="tk")
                nc.vector.tensor_scalar(m01, Ct, CR_KEEP, None, op0=ALU.is_lt)
                # excl = (1-m01)*BIG via exact {0,+-BIG} arithmetic (m01*(-BIG)+BIG);
                # a naive m01*(bc-BIG)+BIG absorbs bc into BIG and loses it.
                t2 = tkp.tile([1, COUT], F32, name=f"t2{b}", tag="tk")
                nc.vector.tensor_scalar(t2, m01, -BIG, BIG,
                                        op0=ALU.mult, op1=ALU.add)
                t3 = tkp.tile([1, COUT], F32, name=f"t3{b}", tag="tk")
                nc.vector.tensor_tensor(out=t3, in0=m01, in1=bc[0:1, :],
                                        op=ALU.mult)
                sel = tkp.tile([1, COUT], F32, name=f"sel{b}", tag="tk")
                nc.vector.tensor_tensor(out=sel, in0=t3, in1=t2, op=ALU.add)
                thrb = evp.tile([1, 1], F32, name=f"thrb{b}", tag="thrb")
                nc.vector.tensor_reduce(thrb, sel, axis=AX.X, op=ALU.min)
                nc.sync.dma_start(out=thr_d[b, :], in_=thrb)
                if debug:
                    nc.sync.dma_start(out=dbg_cnt[b:b + 1, :], in_=Ct)
                    nc.sync.dma_start(out=dbg_bc[b:b + 1, :], in_=bc[0:1, :])

            thrc = pp.tile([NB, 1], F32, name="thrc")
            nc.sync.dma_start(out=thrc, in_=thr_d)
            if debug:
                nc.sync.dma_start(out=dbg_thr[:, :], in_=thrc)
            gtm = pp.tile([NB, COUT], F32, name="gtm")
            nc.vector.tensor_scalar(gtm, salT, thrc[:, :], None, op0=ALU.is_gt)
            maskT = pp.tile([NB, COUT], F32, name="maskT")
            nc.vector.tensor_tensor(out=maskT, in0=gtm, in1=salT, op=ALU.mult)
            if debug:
                nc.sync.dma_start(out=dbg_mask[:, :], in_=maskT)
                nc.sync.dma_start(out=dbg_sal[:, :], in_=salT)
                for t in range(NT):
                    nc.sync.dma_start(out=dbg_sub[:, t * NB:(t + 1) * NB], in_=subm[t])
                    nc.sync.dma_start(out=dbg_wq[t, :, :], in_=wq[t].bitcast(F32))
            mask_cb = []
            for cot in range(NCOT):
                ptr3 = psS.tile([128, 512], F32, name=f"ptm{cot}", tag="small")
                nc.tensor.transpose(ptr3[0:128, 0:NB],
                                    maskT[:, cot * 128:(cot + 1) * 128],
                                    ident[0:NB, 0:NB])
                mc = pp.tile([128, NB], F32, name=f"mcb{cot}")
                nc.scalar.copy(mc, ptr3[0:128, 0:NB])
                mask_cb.append(mc)

            # ---------- conv: 32 accumulating f32r matmuls per output tile ----------
            y_d = dp.tile([NB, NCOT, 128, NSP], F32, name="y_d")
            s1 = [pp.tile([128, 2 * NB], F32, name=f"s1_{c}") for c in range(NCOT)]
            s2 = [pp.tile([128, 2 * NB], F32, name=f"s2_{c}") for c in range(NCOT)]

            for pair in range(2):
                for cot in range(NCOT):
                    banks = [[psA.tile([128, 512], F32, name=f"bk{pair}{cot}{b01}{n}",
                                       tag="conv") for n in range(2)]
                             for b01 in range(2)]
                    n_mm = 0
                    total_mm = NT * 16 * 4
                    for t in range(NT):
                        for kh, kw in KHW_ORDER:
                            ph, dj = PAR[kh]
                            pw, di = PAR[kw]
                            lhsT = wq[t][:, (kh * KK + kw) * COUT + cot * 128:
                                         (kh * KK + kw) * COUT + cot * 128 + 128]
                            for b01 in range(2):
                                for n in range(2):
                                    oh_lo = max(16 * n, -dj)
                                    oh_hi = min(16 * n + 15, OH - 1 - dj)
                                    rhs = quads[b01][t][ph][pw][
                                        :, oh_lo + dj: oh_hi + dj + 1,
                                        1 + di: 1 + di + OW]
                                    dst = banks[b01][n].rearrange(
                                        "p (r c) -> p r c", r=16)[
                                        :, oh_lo - 16 * n: oh_hi - 16 * n + 1, :]
                                    nc.tensor.matmul(
                                        dst, lhsT, rhs,
                                        start=(n_mm < 4), stop=(n_mm >= total_mm - 4),
                                        skip_group_check=True)
                                    n_mm += 1
                    for b01 in range(2):
                        b = pair * 2 + b01
                        for n in range(2):
                            yt = evp.tile([128, 512], F32, name=f"yt{pair}{cot}{b01}{n}",
                                          tag="yt")
                            slot = pair * 4 + b01 * 2 + n
                            nc.scalar.activation(
                                yt, banks[b01][n], AF.Copy, bias=0.0,
                                scale=mask_cb[cot][:, b:b + 1],
                                accum_out=s1[cot][:, slot:slot + 1])
                            nc.sync.dma_start(
                                out=y_d[b, cot, :, n * 512:(n + 1) * 512], in_=yt)
                            sq = psS.tile([128, 512], F32, name=f"sq{pair}{cot}{b01}{n}",
                                          tag="small")
                            nc.scalar.activation(
                                sq, yt, AF.Square, bias=0.0, scale=1.0,
                                accum_out=s2[cot][:, slot:slot + 1])
                if pair == 0:
                    for b in (2, 3):
                        load_sample(b, False, True)

            # ---------- BN stats allreduce + scale/shift ----------
            cc_in = dp.tile([2, NCOT, 128], F32, name="cc_in")
            for cot in range(NCOT):
                r1 = evp.tile([128, 1], F32, name=f"r1_{cot}", tag="subh", bufs=4)
                nc.vector.tensor_reduce(r1, s1[cot], axis=AX.X, op=ALU.add)
                nc.sync.dma_start(out=cc_in[0, cot, :], in_=r1[:, 0:1])
                r2 = evp.tile([128, 1], F32, name=f"r2_{cot}", tag="subh", bufs=4)
                nc.vector.tensor_reduce(r2, s2[cot], axis=AX.X, op=ALU.add)
                nc.sync.dma_start(out=cc_in[1, cot, :], in_=r2[:, 0:1])
            nc.gpsimd.collective_compute(
                "AllReduce", ALU.add,
                replica_groups=[list(range(N_CORES))],
                ins=[cc_in[:, :, :]], outs=[cc_out[:, :, :]])

            inv_n = 1.0 / float(B * NSP)
            s1t = pp.tile([128, NCOT], F32, name="s1t")
            nc.sync.dma_start(
                out=s1t, in_=bass.AP(tensor=cc_out, offset=0,
                                     ap=[[1, 128], [128, NCOT]]))
            s2t = pp.tile([128, NCOT], F32, name="s2t")
            nc.sync.dma_start(
                out=s2t, in_=bass.AP(tensor=cc_out, offset=NCOT * 128,
                                     ap=[[1, 128], [128, NCOT]]))
            mu = pp.tile([128, NCOT], F32, name="mu")
            nc.vector.tensor_scalar(mu, s1t, inv_n, None, op0=ALU.mult)
            m2 = pp.tile([128, NCOT], F32, name="m2")
            nc.vector.tensor_scalar(m2, s2t, inv_n, None, op0=ALU.mult)
            musq = pp.tile([128, NCOT], F32, name="musq")
            nc.vector.tensor_tensor(out=musq, in0=mu, in1=mu, op=ALU.mult)
            var = pp.tile([128, NCOT], F32, name="var")
            nc.vector.tensor_tensor(out=var, in0=m2, in1=musq, op=ALU.subtract)
            epst = pp.tile([128, 1], F32, name="epst")
            nc.vector.memset(epst, float(eps_imm))
            sv = pp.tile([128, NCOT], F32, name="sv")
            nc.scalar.activation(sv, var, AF.Sqrt, bias=epst[:, :], scale=1.0)
            rstd = pp.tile([128, NCOT], F32, name="rstd")
            nc.vector.reciprocal(rstd, sv)
            scl = pp.tile([128, NCOT], F32, name="scl")
            nc.vector.tensor_tensor(out=scl, in0=gam_t, in1=rstd, op=ALU.mult)
            mscl = pp.tile([128, NCOT], F32, name="mscl")
            nc.vector.tensor_tensor(out=mscl, in0=mu, in1=scl, op=ALU.mult)
            shf = pp.tile([128, NCOT], F32, name="shf")
            nc.vector.tensor_tensor(out=shf, in0=bet_t, in1=mscl, op=ALU.subtract)

            # ---------- epilogue: out = Prelu(y*scale + shift, 0.2) ----------
            for cot in range(NCOT):
                for bp in range(2):
                    ya = stp.tile([128, 2, NSP], F32, name=f"ya{cot}{bp}",
                                  tag="stage")
                    nc.sync.dma_start(
                        out=ya,
                        in_=y_d[bp * 2:(bp + 1) * 2, cot, :, :].rearrange(
                            "b p s -> p b s"))
                    for b01 in range(2):
                        b = bp * 2 + b01
                        for n in range(2):
                            ot = evp.tile([128, 512], F32, name=f"ot{cot}{b}{n}",
                                          tag="yt")
                            nc.scalar.activation(
                                ot, ya[:, b01, n * 512:(n + 1) * 512], AF.Prelu,
                                bias=shf[:, cot:cot + 1], scale=scl[:, cot:cot + 1],
                                alpha=float(NEG_SLOPE))
                            nc.sync.dma_start(
                                out=out[b, cot * 128:(cot + 1) * 128, :, :].rearrange(
                                    "p h w -> p (h w)")[:, n * 512:(n + 1) * 512],
                                in_=ot)

            if debug:
                for b in range(NB):
                    for cc_ in range(NCOT):
                        dcp = stp.tile([128, NSP], F32, name=f"dcp{b}{cc_}", tag="stage")
                        nc.sync.dma_start(out=dcp, in_=y_d[b, cc_, :, :])
                        nc.sync.dma_start(out=dbg_y[b, cc_, :, :], in_=dcp)
                dst_ = stp.tile([128, 2 * NCOT], F32, name="dst_", tag="stage")
                nc.sync.dma_start(out=dst_, in_=bass.AP(tensor=cc_out, offset=0, ap=[[1, 128], [128, 2 * NCOT]]))
                nc.sync.dma_start(out=bass.AP(tensor=dbg_stats, offset=0, ap=[[1, 128], [128, 2 * NCOT]]), in_=dst_)

    _split_waits(nc)
    return nc


_CACHE = {}


def kernel(x, weight, pos, neg, sal_w, sal_b, gamma, beta):
    x = np.ascontiguousarray(np.asarray(x, dtype=np.float32))
    weight = np.ascontiguousarray(np.asarray(weight, dtype=np.float32))
    sal_w = np.ascontiguousarray(np.asarray(sal_w, dtype=np.float32))
    sal_b = np.ascontiguousarray(np.asarray(sal_b, dtype=np.float32))
    gamma = np.ascontiguousarray(np.asarray(gamma, dtype=np.float32))
    beta = np.ascontiguousarray(np.asarray(beta, dtype=np.float32))
    pos_f = np.float32(np.asarray(pos).reshape(()))
    neg_f = np.float32(np.asarray(neg).reshape(()))

    r_imm = float(np.float32(neg_f / pos_f))
    eps_imm = float(np.float32(BN_EPS) / (pos_f * pos_f))

    import os
    debug = os.environ.get("KERNEL_DEBUG", "0") == "1"
    key = (r_imm, eps_imm, debug)
    if key not in _CACHE:
        _CACHE[key] = build_kernel(r_imm, eps_imm, debug)
    nc = _CACHE[key]

    in_maps = []
    for c in range(N_CORES):
        in_maps.append({
            "xs": x[c * NB:(c + 1) * NB],
            "wt": weight,
            "salw": sal_w,
            "salb": sal_b,
            "gam": gamma,
            "bet": beta,
        })
    res = run_bass_kernel_spmd(nc, in_maps, core_ids=list(range(N_CORES)))
    if debug:
        kernel.dbg = res.results
    out = np.concatenate([res.results[c]["out"] for c in range(N_CORES)], axis=0)
    return out



# revision 12
# speedup vs baseline: 2.0251x; 2.0251x over previous
"""Trainium2 Bass kernel for nn_BasicBlock (FBS-masked ternary conv + BN + LeakyReLU).

Sharding: data-parallel over batch. B=32 -> 4 samples per core on 8 cores.

Key ideas vs a straightforward f32r implementation:
  - The ternary weights take only 3 values {pos, 0, neg}.  We pick fp8-e4m3-
    exact u, v with v/u ~ r = neg/pos (best ratio pair, ~0.3% error) and run
    the conv entirely in fp8 with MatmulPerfMode.DoubleRow (0.5 cycles/row,
    256-wide contraction): both ci tiles are packed into the two DoubleRow
    halves.  x is split x = hi + lo (e4m3 each); hi and lo passes accumulate
    into the same PSUM banks, recovering ~bf16 accuracy (~0.3% overall).
  - Conv output y' = (u/pos) * y_true.  BN absorbs the scale except through
    eps: eps' = eps * (u/pos)^2 (host immediate).
  - Quadrant decomposition for the stride-2 K=4 conv: 4 parity quadrants,
    33-wide rows (one shared zero pad col) so every tap is a single
    contiguous flat run per PSUM bank (rank-3 APs for DoubleRow).
  - y stays in SBUF as bf16 (no DRAM round trip); the FBS mask is folded
    into the per-(sample, channel-tile) epilogue scale; BN batch stats go
    through one AllGather (cheaper than AllReduce in practice) + local sum.
  - Top-k threshold (k=410 of 512) is exact: count matrix via compare +
    ones-matmul, thr = min{s_j : #{s_i > s_j} <= 409}.
"""

import numpy as np
import ml_dtypes

import concourse.bass as bass
import concourse.mybir as mybir
import concourse.tile as tile
from concourse.bass_utils import run_bass_kernel_spmd
from concourse.masks import make_identity

F32 = mybir.dt.float32
BF16 = mybir.dt.bfloat16
F8 = mybir.dt.float8e4
AF = mybir.ActivationFunctionType
ALU = mybir.AluOpType
AX = mybir.AxisListType
DR = mybir.MatmulPerfMode.DoubleRow

N_CORES = 8
B, CIN, H, W = 32, 256, 64, 64
COUT, KK = 512, 4
OH, OW = 32, 32
NB = B // N_CORES          # samples per core = 4
NT = CIN // 128            # ci tiles = 2
NCOT = COUT // 128         # co tiles = 4
CR_KEEP = 409.5            # count <= 409  <->  count < 409.5
BN_EPS = 1e-5
NEG_SLOPE = 0.2
THRESH_FACTOR = 0.05
NSP = OH * OW              # 1024 spatial positions per sample
W33 = OW + 1               # quad row stride; col 32 is a shared zero pad

MAX_WAITS = 1              # this walrus build allows 1 sync wait per instruction

# kh -> (row parity ph, row shift dj): x row 2*oh + kh - 1 = 2*(oh+dj) + ph
PAR = {0: (1, -1), 1: (0, 0), 2: (1, 0), 3: (0, 1)}
KHW_ORDER = [(1, 1)] + [(kh, kw) for kh in range(KK) for kw in range(KK)
                        if (kh, kw) != (1, 1)]
BIG = 1.0e30


def _split_waits(nc, max_waits=MAX_WAITS):
    """Split per-instruction sem waits exceeding max_waits into preceding
    same-engine InstNoOp carriers (engines execute their queue in order)."""
    for f in nc.m.functions:
        for bb in f.blocks:
            new_list = []
            changed = False
            for ins in bb.instructions:
                si = ins.sync_info
                if si is not None and si.on_wait and len(si.on_wait) > max_waits:
                    waits = list(si.on_wait)
                    carry = waits[: len(waits) - max_waits]
                    keep = waits[len(waits) - max_waits:]
                    k = 0
                    while carry:
                        chunk, carry = carry[:max_waits], carry[max_waits:]
                        new_list.append(
                            mybir.InstNoOp(
                                name=f"{ins.name}_ws{k}",
                                engine=ins.engine,
                                bass_nofuse=True,
                                sync_info=mybir.SyncInfo(on_wait=chunk, on_update=[]),
                            )
                        )
                        k += 1
                    ins.sync_info = mybir.SyncInfo(
                        on_wait=keep, on_update=list(si.on_update)
                    )
                    changed = True
                new_list.append(ins)
            if changed:
                bb.instructions = new_list


def best_fp8_pair(r):
    """e4m3-exact (u, v) minimizing |v/u - r|/|r|."""
    best = None
    for m in range(8, 16):
        for k in range(-3, 4):
            u = m * (2.0 ** k) / 8.0
            v = float(np.float32(u * r).astype(ml_dtypes.float8_e4m3fn)
                      .astype(np.float32))
            if v == 0.0 or abs(v) > 448:
                continue
            err = abs(v / u - r) / abs(r)
            if best is None or err < best[0]:
                best = (err, u, v)
    return best[1], best[2]


def build_kernel(u_imm, v_imm, eps_imm, debug=False, sim_compat=False):
    nc = bass.Bass()

    xs = nc.dram_tensor("xs", [NB, CIN, H, W], F32, kind="ExternalInput")
    wt = nc.dram_tensor("wt", [COUT, CIN, KK, KK], F32, kind="ExternalInput")
    salw = nc.dram_tensor("salw", [COUT, CIN], F32, kind="ExternalInput")
    salb = nc.dram_tensor("salb", [COUT], F32, kind="ExternalInput")
    gam = nc.dram_tensor("gam", [COUT], F32, kind="ExternalInput")
    bet = nc.dram_tensor("bet", [COUT], F32, kind="ExternalInput")
    out = nc.dram_tensor("out", [NB, COUT, OH, OW], F32, kind="ExternalOutput")

    cc_out = nc.dram_tensor("cc_out", [N_CORES, 128, 2 * NCOT], F32,
                            addr_space="Shared")

    if debug:
        dbg_sal = nc.dram_tensor("dbg_sal", [NB, COUT], F32, kind="ExternalOutput")
        dbg_thr = nc.dram_tensor("dbg_thr", [NB, 1], F32, kind="ExternalOutput")
        dbg_mask = nc.dram_tensor("dbg_mask", [NB, COUT], F32, kind="ExternalOutput")
        dbg_y = nc.dram_tensor("dbg_y", [NB, NCOT, 128, NSP], F32,
                               kind="ExternalOutput")
        dbg_st = nc.dram_tensor("dbg_st", [128, 2 * NCOT], F32,
                                kind="ExternalOutput")
        dbg_wq = nc.dram_tensor("dbg_wq", [128, 16 * NT * COUT], F32,
                                kind="ExternalOutput")

    with tile.TileContext(nc) as tc:
        with (
            tc.tile_pool(name="persist", bufs=1) as pp,
            tc.tile_pool(name="wsq", bufs=16) as wqp,
            tc.tile_pool(name="xst", bufs=3) as xsp,
            tc.tile_pool(name="gst", bufs=2) as gsp,
            tc.tile_pool(name="small", bufs=4) as smp,
            tc.tile_pool(name="otp", bufs=2) as otp,
            tc.tile_pool(name="psum", bufs=1, space="PSUM") as psp,
            tc.tile_pool(name="dram", bufs=1, space="DRAM") as dp,
        ):
            # ---------- constants ----------
            ident = pp.tile([128, 128], F32, name="ident")
            make_identity(nc, ident)
            ident8 = pp.tile([128, 128], F8, name="ident8")
            nc.gpsimd.tensor_copy(ident8, ident)
            onesk = pp.tile([128, NB], BF16, name="onesk")
            nc.vector.memset(onesk, 1.0)
            ones1 = pp.tile([1, 128], F32, name="ones1")
            nc.vector.memset(ones1, 1.0)
            ebs = []
            ehot = []
            for b in range(NB):
                eb = pp.tile([NB, 128], F32, name=f"eb{b}")
                nc.gpsimd.memset(eb, 0.0)
                # eb[x, y] = (x == b) ? 1 : 0
                nc.gpsimd.affine_select(
                    out=eb, in_=eb, compare_op=ALU.not_equal, fill=1.0,
                    base=-b, pattern=[[0, 128]], channel_multiplier=1)
                ebs.append(eb)
                eh = pp.tile([128, NB], BF16, name=f"eh{b}")
                nc.gpsimd.memset(eh, 0.0)
                # eh[x, y] = (y == b) ? 1 : 0
                nc.gpsimd.affine_select(
                    out=eh, in_=eh, compare_op=ALU.not_equal, fill=1.0,
                    base=-b, pattern=[[1, NB]], channel_multiplier=0)
                ehot.append(eh)

            def col128(dram_vec, nm):  # [512] dram -> [128,4] sbuf
                t_ = pp.tile([128, NCOT], F32, name=nm)
                ap = bass.AP(tensor=dram_vec, offset=0, ap=[[1, 128], [128, NCOT]])
                nc.sync.dma_start(out=t_, in_=ap)
                return t_

            # ---------- DMA: w (natural layout), saliency consts, then x ----
            wst = []  # 16 subchunks [128co, 1024 = 64ci*16khw]; (c, q)
            mx = pp.tile([128, 16], F32, name="mx")
            for c in range(NCOT):
                for q in range(4):
                    ws_ = wqp.tile([128, 1024], F32, name=f"ws{c}{q}",
                                   tag="wsq", padded_shape=[128, 1056])
                    nc.sync.dma_start(
                        out=ws_,
                        in_=wt[c * 128:(c + 1) * 128,
                               q * 64:(q + 1) * 64, :, :].rearrange(
                                   "co ci kh kw -> co (ci kh kw)"))
                    wst.append(ws_)
                    nc.vector.tensor_reduce(
                        mx[:, c * 4 + q: c * 4 + q + 1], ws_, axis=AX.X,
                        op=ALU.max, apply_absolute_value=True)

            salb_t = col128(salb, "salb_t")
            gam_t = col128(gam, "gam_t")
            bet_t = col128(bet, "bet_t")
            swn = []
            for c in range(NCOT):
                sw_ = pp.tile([128, CIN], F32, name=f"swn{c}")
                nc.sync.dma_start(out=sw_, in_=salw[c * 128:(c + 1) * 128, :])
                swn.append(sw_)

            # x half-sample stages; order b-major so quads complete in order
            stg = {}
            for b in range(NB):
                for t in range(NT):
                    for hh in range(2):
                        s_ = xsp.tile([128, 32, W], F32, name=f"x{b}{t}{hh}",
                                      tag="x")
                        nc.sync.dma_start(
                            out=s_,
                            in_=xs[b, t * 128:(t + 1) * 128,
                                   hh * 32:(hh + 1) * 32, :])
                        stg[(b, t, hh)] = s_

            # ---------- global max |w| -> tcol/ntcol ----------
            mxr = pp.tile([128, 1], F32, name="mxr")
            nc.vector.tensor_reduce(mxr, mx, axis=AX.X, op=ALU.max)
            ps_s = psp.tile([128, 512], F32, name="ps_g", tag="small", bufs=2)
            nc.tensor.transpose(ps_s[0:1, 0:128], mxr, ident)
            gmaxrow = pp.tile([1, 128], F32, name="gmaxrow")
            nc.scalar.copy(gmaxrow, ps_s[0:1, 0:128])
            gmax = pp.tile([1, 1], F32, name="gmax")
            nc.vector.tensor_reduce(gmax, gmaxrow, axis=AX.X, op=ALU.max)
            ps_b = psp.tile([128, 512], F32, name="ps_b", tag="small", bufs=2)
            nc.tensor.matmul(ps_b[:, 0:1], ones1, gmax, start=True, stop=True)
            tcol = pp.tile([128, 1], F32, name="tcol")
            nc.scalar.activation(tcol, ps_b[:, 0:1], AF.Copy, bias=0.0,
                                 scale=float(THRESH_FACTOR))
            ntcol = pp.tile([128, 1], F32, name="ntcol")
            nc.scalar.activation(ntcol, ps_b[:, 0:1], AF.Copy, bias=0.0,
                                 scale=-float(THRESH_FACTOR))

            # ---------- ternarize + transpose into DoubleRow lhsT layout ----
            # wq[p=ci%128, khw, t, co] fp8; lhsT slice [128, 2, 128] per (khw, cot)
            wq = pp.tile([128, 16, NT, COUT], F8, name="wq")
            for c in range(NCOT):          # co chunk (c-major: conv needs c0 first)
                for t in range(NT):
                    gt = gsp.tile([128, 2048], F8, name=f"gt{c}{t}", tag="gt")
                    for qq in range(2):    # the two 64-ci subchunks of this t
                        q = t * 2 + qq
                        wsrc = wst[c * 4 + q]
                        g1 = gsp.tile([128, 1024], F8, name=f"g1_{c}{q}", tag="g1")
                        nc.vector.tensor_scalar(g1, wsrc, tcol[:, :],
                                                float(u_imm),
                                                op0=ALU.is_gt, op1=ALU.mult)
                        g2 = gsp.tile([128, 1024], F8, name=f"g2_{c}{q}", tag="g2")
                        eng2 = nc.gpsimd if (q % 2 == 0) else nc.vector
                        eng2.tensor_scalar(g2, wsrc, ntcol[:, :], float(v_imm),
                                           op0=ALU.is_lt, op1=ALU.mult)
                        enga = nc.vector if (q % 2 == 0) else nc.gpsimd
                        enga.tensor_tensor(out=gt[:, qq * 1024:(qq + 1) * 1024],
                                           in0=g1, in1=g2, op=ALU.add)
                    # transpose [co128, ci128] blocks per khw into psum
                    ps_t = psp.tile([128, 2048], F8, name=f"pt{c}{t}",
                                    tag="small", bufs=2)
                    for khw in range(16):
                        src = bass.AP(tensor=gt.tensor, offset=gt.offset + khw,
                                      ap=[gt.ap[0], [16, 128]])
                        nc.tensor.transpose(ps_t[:, khw * 128:(khw + 1) * 128],
                                            src, ident8)
                    dst = wq[:, :, t, c * 128:(c + 1) * 128]
                    engc = nc.scalar if (c % 2 == 0) else nc.gpsimd
                    if engc is nc.scalar:
                        nc.scalar.copy(dst, ps_t)
                    else:
                        nc.gpsimd.tensor_copy(dst, ps_t)

            if debug:
                wq32 = pp.tile([128, 16 * NT * COUT], F32, name="wq32")
                nc.vector.tensor_copy(
                    wq32, wq.rearrange("p a t c -> p (a t c)"))
                nc.sync.dma_start(out=dbg_wq[:, :], in_=wq32)

            # ---------- quad tiles (reuse w-chunk slots) + pad zeroing ----
            # Q[hl][b][ph] = [128, 2(pw), 2(t), 32(oh), 33]; col 32 is zero pad
            # creation order b-major so Q(b) lands on the slot of w chunk c=b,
            # whose ternarize completes before sample b's data arrives.
            Q = [[[None for ph in range(2)] for b in range(NB)] for hl in range(2)]
            for b in range(NB):
                for hl in range(2):
                    for ph in range(2):
                        qt_ = wqp.tile([128, 2, NT, OH, W33], F8,
                                       name=f"q{hl}{b}{ph}", tag="wsq")
                        nc.gpsimd.memset(qt_[:, :, :, :, OW:W33], 0.0)
                        Q[hl][b][ph] = qt_

            # ---------- salw transposes (PE, pre-conv) ----------
            salwT = [pp.tile([128, COUT], F32, name=f"swT{t}") for t in range(NT)]
            for c in range(NCOT):
                for t in range(NT):
                    ps_w = psp.tile([128, 512], F32, name=f"pw{c}{t}",
                                    tag="small", bufs=2)
                    nc.tensor.transpose(ps_w[:, 0:128],
                                        swn[c][:, t * 128:(t + 1) * 128], ident)
                    nc.scalar.copy(salwT[t][:, c * 128:(c + 1) * 128],
                                   ps_w[:, 0:128])

            # ---------- x quantize + interleave + |x| sums ----------
            subT = [pp.tile([128, NB], F32, name=f"subT{t}") for t in range(NT)]
            subh = smp.tile([128, NT * NB * 2], F32, name="subh", tag="subh",
                            bufs=1)

            def interleave(b, t, hh):
                s_ = stg[(b, t, hh)]
                # |x| partial sums on ACT (scratch) for hh=0, DVE reduce hh=1
                col = (t * NB + b) * 2 + hh
                if hh == 0:
                    scr = xsp.tile([128, 32 * W], F32, name=f"sc{b}{t}",
                                   tag="scr", bufs=2)
                    nc.scalar.activation(scr, s_.rearrange("p a b -> p (a b)"),
                                         AF.Abs, bias=0.0, scale=1.0,
                                         accum_out=subh[:, col:col + 1])
                else:
                    nc.vector.tensor_reduce(subh[:, col:col + 1], s_,
                                            axis=AX.XY, op=ALU.add,
                                            apply_absolute_value=True)
                for ph in range(2):
                    # in: stg rows ph::2 viewed (pw, r, w); out: Q[., pw, t, hh16+r, w]
                    src = bass.AP(tensor=s_.tensor, offset=s_.offset + ph * W,
                                  ap=[s_.ap[0], [1, 2], [2 * W, 16], [2, OW]])
                    qhi = Q[0][b][ph][:, :, t, hh * 16:(hh + 1) * 16, 0:OW]
                    nc.scalar.copy(qhi, src)
                    qlo = Q[1][b][ph][:, :, t, hh * 16:(hh + 1) * 16, 0:OW]
                    nc.vector.tensor_tensor(out=qlo, in0=src, in1=qhi,
                                            op=ALU.subtract)

            for b in (0, 1):
                for t in range(NT):
                    for hh in range(2):
                        interleave(b, t, hh)

            # ---------- conv ----------
            y = [[pp.tile([128, NSP], BF16, name=f"y{b}{c}") for c in range(NCOT)]
                 for b in range(NB)]
            s1 = [pp.tile([128, 2 * NB], F32, name=f"s1_{c}") for c in range(NCOT)]
            s2 = [pp.tile([128, 2 * NB], F32, name=f"s2_{c}") for c in range(NCOT)]

            def conv_group(b, cot):
                halves = [psp.tile([128, 1024], F32, name=f"bk{b}{cot}{h}",
                                   tag="conv", bufs=3) for h in range(2)]
                n_mm = 0
                total = 2 * 16 * 8
                for hl in range(2):
                    for kh, kw in KHW_ORDER:
                        ph, dj = PAR[kh]
                        pw, di = PAR[kw]
                        lhsT = wq[:, kh * KK + kw, :, cot * 128:(cot + 1) * 128]
                        qt = Q[hl][b][ph]
                        for h in range(2):
                            for j in range(2):
                                r0 = 16 * h + 8 * j
                                rows_lo = max(r0, -dj)
                                rows_hi = min(r0 + 7, OH - 1 - dj)
                                d0 = 512 * j + (rows_lo - r0) * W33
                                s0 = pw * (NT * OH * W33) + \
                                    (rows_lo + dj) * W33 + di
                                L = (rows_hi - rows_lo) * W33 + W33
                                if s0 < pw * (NT * OH * W33):
                                    d0 += 1
                                    s0 += 1
                                    L -= 1
                                rhs = bass.AP(
                                    tensor=qt.tensor, offset=qt.offset + s0,
                                    ap=[qt.ap[0], [OH * W33, NT], [1, L]])
                                nc.tensor.matmul(
                                    halves[h][:, d0:d0 + L], lhsT, rhs,
                                    start=(n_mm < 4), stop=(n_mm >= total - 4),
                                    perf_mode=DR, skip_group_check=True)
                                n_mm += 1
                for h in range(2):
                    src = bass.AP(tensor=halves[h].tensor,
                                  offset=halves[h].offset,
                                  ap=[halves[h].ap[0], [512, 2], [W33, 8],
                                      [1, OW]])
                    slot = b * 2 + h
                    nc.scalar.activation(
                        y[b][cot][:, h * 512:(h + 1) * 512], src, AF.Copy,
                        bias=0.0, scale=1.0,
                        accum_out=s1[cot][:, slot:slot + 1])
                    sq = smp.tile([128, 512], F32, name=f"sq{b}{cot}{h}",
                                  tag="sq", bufs=2)
                    nc.vector.tensor_tensor_reduce(
                        out=sq, in0=y[b][cot][:, h * 512:(h + 1) * 512],
                        in1=y[b][cot][:, h * 512:(h + 1) * 512],
                        scale=1.0, scalar=0.0, op0=ALU.mult, op1=ALU.add,
                        accum_out=s2[cot][:, slot:slot + 1])

            for cot in range(NCOT):
                conv_group(0, cot)
            for t in range(NT):
                for hh in range(2):
                    interleave(2, t, hh)
            for cot in range(NCOT):
                conv_group(1, cot)
            for t in range(NT):
                for hh in range(2):
                    interleave(3, t, hh)

            # ---------- saliency + top-k (emitted mid-conv; deps ready) ----
            subm = [pp.tile([128, NB], F32, name=f"subm{t}") for t in range(NT)]
            for t in range(NT):
                sview = bass.AP(tensor=subh.tensor,
                                offset=subh.offset + t * NB * 2,
                                ap=[subh.ap[0], [2, NB], [1, 2]])
                nc.vector.tensor_reduce(subT[t], sview, axis=AX.X, op=ALU.add)
                nc.vector.tensor_scalar(subm[t], subT[t], 1.0 / (H * W), None,
                                        op0=ALU.mult)
            sal_cb = []
            for cot in range(NCOT):
                ps_sal = psp.tile([128, 512], F32, name=f"psal{cot}",
                                  tag="small", bufs=2)
                for t in range(NT):
                    nc.tensor.matmul(ps_sal[:, 0:NB],
                                     salwT[t][:, cot * 128:(cot + 1) * 128],
                                     subm[t], start=(t == 0), stop=(t == NT - 1))
                sc = pp.tile([128, NB], F32, name=f"salcb{cot}")
                nc.scalar.activation(sc, ps_sal[:, 0:NB], AF.Abs,
                                     bias=salb_t[:, cot:cot + 1], scale=1.0)
                sal_cb.append(sc)
            salT = pp.tile([NB, COUT], F32, name="salT")
            for cot in range(NCOT):
                ps_st = psp.tile([128, 512], F32, name=f"pst{cot}",
                                 tag="small", bufs=2)
                nc.tensor.transpose(ps_st[0:NB, 0:128], sal_cb[cot], ident)
                nc.scalar.copy(salT[:, cot * 128:(cot + 1) * 128],
                               ps_st[0:NB, 0:128])
            if debug:
                nc.sync.dma_start(out=dbg_sal[:, :], in_=salT)

            # counts: CtAll[b, j] = #{i: sal[b,i] > sal[b,j]}; one-hot lhsT
            # routes sample b's counts to psum row b, accumulated across b.
            CtAll = pp.tile([NB, COUT], F32, name="CtAll")
            ps_c = psp.tile([128, 512], F32, name="pc", tag="small", bufs=2)
            for b in range(NB):
                ps_bc = psp.tile([128, 512], F32, name=f"pbc{b}",
                                 tag="small", bufs=2)
                nc.tensor.matmul(ps_bc[:, 0:COUT], ebs[b], salT,
                                 start=True, stop=True)
                bc = smp.tile([128, COUT], F32, name=f"bc{b}", tag="bc", bufs=2)
                nc.scalar.copy(bc, ps_bc[:, 0:COUT])
                for cot in range(NCOT):
                    cmp = smp.tile([128, COUT], BF16, name=f"cmp{b}{cot}",
                                   tag="cmp", bufs=2)
                    nc.vector.tensor_scalar(cmp, bc, sal_cb[cot][:, b:b + 1],
                                            None, op0=ALU.is_lt)
                    nc.tensor.matmul(ps_c[0:NB, :], ehot[b], cmp,
                                     start=(b == 0 and cot == 0),
                                     stop=(b == NB - 1 and cot == NCOT - 1),
                                     skip_group_check=True)
            nc.scalar.copy(CtAll, ps_c[0:NB, :])

            # thr[b] = min{sal : count <= 409}, all 4 samples in one [4, 512]
            m01 = smp.tile([NB, COUT], F32, name="m01", tag="tk", bufs=3)
            nc.vector.tensor_scalar(m01, CtAll, CR_KEEP, None, op0=ALU.is_lt)
            t2 = smp.tile([NB, COUT], F32, name="t2", tag="tk", bufs=3)
            nc.vector.tensor_scalar(t2, m01, -BIG, BIG, op0=ALU.mult,
                                    op1=ALU.add)
            t3 = smp.tile([NB, COUT], F32, name="t3", tag="tk", bufs=3)
            nc.vector.tensor_tensor(out=t3, in0=m01, in1=salT, op=ALU.mult)
            sel = smp.tile([NB, COUT], F32, name="sel", tag="tk", bufs=3)
            nc.vector.tensor_tensor(out=sel, in0=t3, in1=t2, op=ALU.add)
            thrc = pp.tile([NB, 1], F32, name="thrc")
            nc.vector.tensor_reduce(thrc, sel, axis=AX.X, op=ALU.min)
            if debug:
                nc.sync.dma_start(out=dbg_thr[:, :], in_=thrc)
            gtm = smp.tile([NB, COUT], F32, name="gtm", tag="tk", bufs=3)
            nc.vector.tensor_scalar(gtm, salT, thrc[:, :], None, op0=ALU.is_gt)
            maskT = pp.tile([NB, COUT], F32, name="maskT")
            nc.vector.tensor_tensor(out=maskT, in0=gtm, in1=salT, op=ALU.mult)
            if debug:
                nc.sync.dma_start(out=dbg_mask[:, :], in_=maskT)
            mask_cb, msq_cb = [], []
            for cot in range(NCOT):
                ps_m = psp.tile([128, 512], F32, name=f"pm{cot}",
                                tag="small", bufs=2)
                nc.tensor.transpose(ps_m[0:128, 0:NB],
                                    maskT[:, cot * 128:(cot + 1) * 128],
                                    ident[0:NB, 0:NB])
                mc = pp.tile([128, NB], F32, name=f"mcb{cot}")
                nc.scalar.copy(mc, ps_m[0:128, 0:NB])
                mask_cb.append(mc)
                mq = pp.tile([128, NB], F32, name=f"msq{cot}")
                nc.gpsimd.tensor_tensor(out=mq, in0=mc, in1=mc, op=ALU.mult)
                msq_cb.append(mq)

            # ---------- conv pair 1 ----------
            for b in (2, 3):
                for cot in range(NCOT):
                    conv_group(b, cot)

            # ---------- masked BN stats + AllGather + scale/shift ----------
            cs = pp.tile([128, 2 * NCOT], F32, name="cs")
            for cot in range(NCOT):
                s1b = pp.tile([128, NB], F32, name=f"s1b{cot}")
                a0 = bass.AP(tensor=s1[cot].tensor, offset=s1[cot].offset,
                             ap=[s1[cot].ap[0], [2, NB], [1, 2]])
                nc.vector.tensor_reduce(s1b, a0, axis=AX.X, op=ALU.add)
                s2b = pp.tile([128, NB], F32, name=f"s2b{cot}")
                a1 = bass.AP(tensor=s2[cot].tensor, offset=s2[cot].offset,
                             ap=[s2[cot].ap[0], [2, NB], [1, 2]])
                nc.vector.tensor_reduce(s2b, a1, axis=AX.X, op=ALU.add)
                w1 = pp.tile([128, NB], F32, name=f"w1{cot}")
                nc.vector.tensor_tensor(out=w1, in0=s1b, in1=mask_cb[cot],
                                        op=ALU.mult)
                nc.vector.tensor_reduce(cs[:, cot:cot + 1], w1, axis=AX.X,
                                        op=ALU.add)
                w2 = pp.tile([128, NB], F32, name=f"w2{cot}")
                nc.vector.tensor_tensor(out=w2, in0=s2b, in1=msq_cb[cot],
                                        op=ALU.mult)
                nc.vector.tensor_reduce(cs[:, NCOT + cot:NCOT + cot + 1], w2,
                                        axis=AX.X, op=ALU.add)

            cc_in = dp.tile([128, 2 * NCOT], F32, name="cc_in")
            nc.sync.dma_start(out=cc_in, in_=cs)
            nc.gpsimd.collective_compute(
                "AllGather", ALU.bypass,
                replica_groups=[list(range(N_CORES))],
                ins=[cc_in[:, :]], outs=[cc_out[:, :, :]])
            gth = pp.tile([128, N_CORES, 2 * NCOT], F32, name="gth")
            nc.sync.dma_start(
                out=gth,
                in_=bass.AP(tensor=cc_out, offset=0,
                            ap=[[2 * NCOT, 128], [128 * 2 * NCOT, N_CORES],
                                [1, 2 * NCOT]]))
            st = pp.tile([128, 2 * NCOT], F32, name="st")
            nc.vector.tensor_reduce(
                st, bass.AP(tensor=gth.tensor, offset=gth.offset,
                            ap=[gth.ap[0], [1, 2 * NCOT], [2 * NCOT, N_CORES]]),
                axis=AX.X, op=ALU.add)
            if debug:
                nc.sync.dma_start(out=dbg_st[:, :], in_=st)

            inv_n = 1.0 / float(B * NSP)
            mu = pp.tile([128, NCOT], F32, name="mu")
            nc.vector.tensor_scalar(mu, st[:, 0:NCOT], inv_n, None,
                                    op0=ALU.mult)
            m2 = pp.tile([128, NCOT], F32, name="m2")
            nc.vector.tensor_scalar(m2, st[:, NCOT:2 * NCOT], inv_n, None,
                                    op0=ALU.mult)
            musq = pp.tile([128, NCOT], F32, name="musq")
            nc.vector.tensor_tensor(out=musq, in0=mu, in1=mu, op=ALU.mult)
            var = pp.tile([128, NCOT], F32, name="var")
            nc.vector.tensor_tensor(out=var, in0=m2, in1=musq, op=ALU.subtract)
            epst = pp.tile([128, 1], F32, name="epst")
            nc.vector.memset(epst, float(eps_imm))
            sv = pp.tile([128, NCOT], F32, name="sv")
            nc.scalar.activation(sv, var, AF.Sqrt, bias=epst[:, :], scale=1.0)
            rstd = pp.tile([128, NCOT], F32, name="rstd")
            nc.vector.reciprocal(rstd, sv)
            scl = pp.tile([128, NCOT], F32, name="scl")
            nc.vector.tensor_tensor(out=scl, in0=gam_t, in1=rstd, op=ALU.mult)
            mscl = pp.tile([128, NCOT], F32, name="mscl")
            nc.vector.tensor_tensor(out=mscl, in0=mu, in1=scl, op=ALU.mult)
            shf = pp.tile([128, NCOT], F32, name="shf")
            nc.vector.tensor_tensor(out=shf, in0=bet_t, in1=mscl,
                                    op=ALU.subtract)

            # ---------- epilogue ----------
            for cot in range(NCOT):
                for b in range(NB):
                    svec = pp.tile([128, 1], F32, name=f"sv{b}{cot}")
                    nc.vector.tensor_tensor(out=svec,
                                            in0=mask_cb[cot][:, b:b + 1],
                                            in1=scl[:, cot:cot + 1],
                                            op=ALU.mult)
                    ot = otp.tile([128, NSP], F32, name=f"ot{b}{cot}",
                                  tag="ot", bufs=1 if sim_compat else 2)
                    if sim_compat:
                        # interp has no Prelu: max(z, 0.2 z) is identical
                        nc.vector.tensor_scalar(ot, y[b][cot], svec[:, :],
                                                shf[:, cot:cot + 1],
                                                op0=ALU.mult, op1=ALU.add)
                        z2 = otp.tile([128, NSP], F32, name=f"z2{b}{cot}",
                                      tag="z2", bufs=1)
                        nc.vector.tensor_scalar(z2, ot, float(NEG_SLOPE),
                                                None, op0=ALU.mult)
                        nc.vector.tensor_tensor(out=ot, in0=ot, in1=z2,
                                                op=ALU.max)
                    else:
                        nc.scalar.activation(ot, y[b][cot], AF.Prelu,
                                             bias=shf[:, cot:cot + 1],
                                             scale=svec[:, :],
                                             alpha=float(NEG_SLOPE))
                    nc.sync.dma_start(
                        out=out[b, cot * 128:(cot + 1) * 128, :, :].rearrange(
                            "p h w -> p (h w)"),
                        in_=ot)
                    if debug:
                        y32 = otp.tile([128, NSP], F32, name=f"yd{b}{cot}",
                                       tag="ot")
                        nc.vector.tensor_copy(y32, y[b][cot])
                        nc.sync.dma_start(out=dbg_y[b, cot, :, :], in_=y32)

    import os
    if os.environ.get("NO_SPLIT_WAITS", "0") != "1":
        _split_waits(nc)
    return nc


_CACHE = {}


def kernel(x, weight, pos, neg, sal_w, sal_b, gamma, beta):
    x = np.ascontiguousarray(np.asarray(x, dtype=np.float32))
    weight = np.ascontiguousarray(np.asarray(weight, dtype=np.float32))
    sal_w = np.ascontiguousarray(np.asarray(sal_w, dtype=np.float32))
    sal_b = np.ascontiguousarray(np.asarray(sal_b, dtype=np.float32))
    gamma = np.ascontiguousarray(np.asarray(gamma, dtype=np.float32))
    beta = np.ascontiguousarray(np.asarray(beta, dtype=np.float32))
    pos_f = float(np.float32(np.asarray(pos).reshape(())))
    neg_f = float(np.float32(np.asarray(neg).reshape(())))

    r = neg_f / pos_f
    u_imm, v_imm = best_fp8_pair(r)
    s = u_imm / pos_f
    eps_imm = float(np.float32(BN_EPS) * s * s)

    import os
    debug = os.environ.get("KERNEL_DEBUG", "0") == "1"
    key = (u_imm, v_imm, eps_imm, debug)
    if key not in _CACHE:
        _CACHE[key] = build_kernel(u_imm, v_imm, eps_imm, debug)
    nc = _CACHE[key]

    in_maps = []
    for c in range(N_CORES):
        in_maps.append({
            "xs": x[c * NB:(c + 1) * NB],
            "wt": weight,
            "salw": sal_w,
            "salb": sal_b,
            "gam": gamma,
            "bet": beta,
        })
    res = run_bass_kernel_spmd(nc, in_maps, core_ids=list(range(N_CORES)))
    if debug:
        kernel.dbg = res.results
    out = np.concatenate([res.results[c]["out"] for c in range(N_CORES)], axis=0)
    return out
